# revision 19
# baseline (speedup 1.0000x reference)
"""Multi-head attention (QKV proj + per-head RMSNorm + softmax attention +
output proj) for Trainium2, distributed over 8 NeuronCores.

Sharding: batch (2) x head-groups (4 heads per core).  Each core computes, for
its batch element and its 4 heads: Q^T/K^T projections (transposed layout,
[d, T]), per-head RMSNorm folded in as a column scale, S^T = K^T.T @ Q^T
scores in [key, query] layout (so softmax normalizers come out of a fused
[V|1] matmul and no P transposes are ever needed), exp (no max subtraction:
|q|=|k|=8 after RMSNorm so |score*0.125| <= 8), O^T = [V|1].T @ P^T, the
normalization, and a partial output projection Out^T = Wo_slice.T @ O^T.
Host sums the 4 partial outputs per batch and transposes back.

All matmuls run in float32r (full fp32 operand bits, PE rounds internally;
1 cycle/row at N>=512 vs 4 for plain fp32), accumulating in fp32 PSUM.
"""

import os
import sys

for _p in ("/opt/trn_rl_repo",):
    if _p not in sys.path:
        sys.path.insert(0, _p)

import numpy as np

B = 2
T = 2048
D = 1024
H = 16
HD = 64
HPC = 4          # heads per core
N_CORES = 8
EPS = 1e-5

_COMPILED = None
LAST_EXEC_NS = None
HOT_BF16 = os.environ.get("ATTN_HOT", "bf16") == "bf16"


def _install_ntff_shim():
    """antenv.axon_hooks is missing in this image; provide it so that
    BASS_TRACE=1 profiling works (mirrors trn_boot's ctypes hook)."""
    import contextlib
    import ctypes
    import types

    if "antenv.axon_hooks" in sys.modules:
        return
    so_path = "/opt/axon/libaxon_pjrt.so"
    if not os.path.exists(so_path):
        return
    lib = ctypes.CDLL(so_path)
    if not hasattr(lib, "axon_start_nrt_profile"):
        return
    lib.axon_start_nrt_profile.argtypes = [ctypes.POINTER(ctypes.c_int64), ctypes.c_size_t]
    lib.axon_start_nrt_profile.restype = ctypes.c_int64
    lib.axon_stop_nrt_profile.argtypes = [ctypes.c_char_p]
    lib.axon_stop_nrt_profile.restype = ctypes.c_int64

    @contextlib.contextmanager
    def _hook(output_dir, device_ids):
        import jax

        jax.devices()
        if device_ids:
            ids = (ctypes.c_int64 * len(device_ids))(*device_ids)
            rc = lib.axon_start_nrt_profile(ids, len(device_ids))
        else:
            rc = lib.axon_start_nrt_profile(None, 0)
        if rc != 0:
            raise RuntimeError(f"axon_start_nrt_profile rc={rc}")
        try:
            yield
        finally:
            n = lib.axon_stop_nrt_profile(str(output_dir).encode())
            print(f"profile: {n} file(s) written to {output_dir}", file=sys.stderr)

    mod = types.ModuleType("antenv.axon_hooks")
    mod._hook = _hook
    mod.get_axon_ntff_profile_hook = lambda: mod._hook
    mod.set_axon_ntff_profile_hook = lambda h: setattr(mod, "_hook", h)
    sys.modules["antenv.axon_hooks"] = mod
    try:
        import antenv

        antenv.axon_hooks = mod
    except ImportError:
        pass


def _build():
    import concourse.bass as bass
    import concourse.tile as tile
    from concourse import bacc, mybir

    F32 = mybir.dt.float32
    F32R = mybir.dt.float32r
    BF16 = mybir.dt.bfloat16
    HOT = BF16 if HOT_BF16 else F32R
    Exp = mybir.ActivationFunctionType.Exp
    Log = mybir.ActivationFunctionType.Ln if hasattr(
        mybir.ActivationFunctionType, "Ln") else mybir.ActivationFunctionType.Log

    TT = T // 128            # 16 t-tiles
    CT = D // 128            # 8 contraction tiles over model dim
    QH = T // 1024           # 2 query halves
    NPAIR = HPC // 2         # 2 head pairs per core

    nc = bacc.Bacc("TRN2", target_bir_lowering=False, debug=False, num_devices=N_CORES)

    HIN = BF16 if HOT_BF16 else F32
    xb = nc.dram_tensor("xb", (T, D), HIN, kind="ExternalInput").ap()
    wq_s = nc.dram_tensor("wq_s", (D, HPC * HD), HIN, kind="ExternalInput").ap()
    wk_s = nc.dram_tensor("wk_s", (D, HPC * HD), HIN, kind="ExternalInput").ap()
    wv_s = nc.dram_tensor("wv_s", (D, HPC * HD), HIN, kind="ExternalInput").ap()
    wo_s = nc.dram_tensor("wo_s", (HPC * HD, D), HIN, kind="ExternalInput").ap()
    ident_d = nc.dram_tensor("ident", (128, 128), HIN, kind="ExternalInput").ap()
    bd2_d = nc.dram_tensor("bd2", (128, 2), F32, kind="ExternalInput").ap()
    wqb_d = nc.dram_tensor("wqb", (128, 128), F32, kind="ExternalInput").ap()
    wkb_d = nc.dram_tensor("wkb", (128, 128), F32, kind="ExternalInput").ap()
    sel_d = [nc.dram_tensor(f"sel{p}", (128, 128), F32, kind="ExternalInput").ap()
             for p in range(NPAIR)]
    onec_d = nc.dram_tensor("onec", (128, 1), HIN, kind="ExternalInput").ap()
    outT = nc.dram_tensor("outT", (D, T), F32, kind="ExternalOutput").ap()

    with tile.TileContext(nc) as tc:
        from contextlib import ExitStack

        with ExitStack() as top:
            # ---- persistent pools -------------------------------------------------
            consts = top.enter_context(tc.tile_pool(name="consts", bufs=1))
            qkpool = top.enter_context(tc.tile_pool(name="qk", bufs=1))
            vppool = top.enter_context(tc.tile_pool(name="vp", bufs=1))

            ident = consts.tile([128, 128], HOT, tag="ident")
            nc.sync.dma_start(out=ident[:], in_=ident_d.bitcast(HOT))
            epsc = consts.tile([128, 1], F32, tag="epsc")
            nc.vector.memset(epsc[:], EPS)
            bd2 = consts.tile([128, 2], F32R, tag="bd2")
            nc.sync.dma_start(out=bd2[:], in_=bd2_d.bitcast(F32R))
            wqb = consts.tile([128, 128], F32R, tag="wqb")
            nc.sync.dma_start(out=wqb[:], in_=wqb_d.bitcast(F32R))
            wkb = consts.tile([128, 128], F32R, tag="wkb")
            nc.sync.dma_start(out=wkb[:], in_=wkb_d.bitcast(F32R))
            sel = []
            for p in range(NPAIR):
                s = consts.tile([128, 128], F32R, tag=f"sel{p}", name=f"sel{p}")
                nc.sync.dma_start(out=s[:], in_=sel_d[p].bitcast(F32R))
                sel.append(s)

            # persistent data tiles
            # per-head tiles, zero-padded to full 128 contraction rows:
            # 64-row matmul weights run ~2x slower than 128-row (no FWL /
            # no LDW overlap), so pad with zeros and contract over 128.
            qhat = [qkpool.tile([128, T], HOT, tag=f"qh{h}", name=f"qhat{h}")
                    for h in range(HPC)]
            khat = [qkpool.tile([128, T], HOT, tag=f"kh{h}", name=f"khat{h}")
                    for h in range(HPC)]
            for h in range(HPC):
                nc.vector.memset(qhat[h][:], 0.0)
                nc.vector.memset(khat[h][:], 0.0)
            # V staging: [128 keys, TT, 2, 65]; [:,tt,h,:] = [V_h|1]
            vp = [vppool.tile([128, TT, 2, 65], HOT, tag=f"vs{p}", name=f"vst{p}")
                  for p in range(NPAIR)]
            for p in range(NPAIR):
                nc.vector.memset(vp[p][:, :, :, 64:65], 1.0)

            # =============== Phase 0+1: X^T, projections, RMS norm ================
            with ExitStack() as p01:
                xinp = p01.enter_context(tc.tile_pool(name="xin", bufs=4))
                xtp = p01.enter_context(tc.tile_pool(name="xT", bufs=CT))
                wpool = p01.enter_context(tc.tile_pool(name="w", bufs=CT))
                qtp = p01.enter_context(tc.tile_pool(name="qt", bufs=4))
                q2p = p01.enter_context(tc.tile_pool(name="q2", bufs=2))
                nsml = p01.enter_context(tc.tile_pool(name="nsml", bufs=1))
                ps_big = p01.enter_context(
                    tc.tile_pool(name="psbig", bufs=3, space="PSUM"))
                ps_sml = p01.enter_context(
                    tc.tile_pool(name="pssml", bufs=2, space="PSUM"))

                # ---- X^T build: 128 PE transposes of [128,128] blocks ----
                xT = [xtp.tile([128, T], HOT, tag="xT", name=f"xT{c}")
                      for c in range(CT)]
                for ttg in range(TT // 4):
                    xts = []
                    for j in range(4):
                        xt = xinp.tile([128, D], HOT, tag="xin")
                        r0 = (ttg * 4 + j) * 128
                        nc.sync.dma_start(out=xt[:], in_=xb[r0:r0 + 128, :].bitcast(HOT))
                        xts.append(xt)
                    for cb in range(CT):
                        tp_ps = ps_big.tile([128, 512], HOT, tag="big")
                        for j in range(4):
                            nc.tensor.transpose(
                                tp_ps[:, j * 128:(j + 1) * 128],
                                xts[j][:, cb * 128:(cb + 1) * 128], ident[:])
                        with nc.allow_low_precision(reason="fp32r rounding"):
                            nc.scalar.copy(
                                xT[cb][:, ttg * 512:(ttg + 1) * 512], tp_ps[:])

                # ---- projections ----
                def project(w_dram, name):
                    """returns per-pair psum eviction targets via callback loop"""
                    wts = []
                    for ct in range(CT):
                        wt = wpool.tile([128, HPC * HD], HOT, tag=f"w{name}",
                                        name=f"w{name}{ct}")
                        nc.sync.dma_start(
                            out=wt[:], in_=w_dram[ct * 128:(ct + 1) * 128, :].bitcast(HOT))
                        wts.append(wt)
                    out_ps = {}
                    for pair in range(NPAIR):
                        for qh in range(QH):
                            pj = ps_big.tile([128, 1024], F32, tag="big")
                            for ct in range(CT):
                                for qq in range(2):
                                    nc.tensor.matmul(
                                        pj[:, qq * 512:(qq + 1) * 512],
                                        wts[ct][:, pair * 128:(pair + 1) * 128],
                                        xT[ct][:, qh * 1024 + qq * 512:
                                               qh * 1024 + (qq + 1) * 512],
                                        start=(ct == 0), stop=(ct == CT - 1))
                            yield pair, qh, pj

                def proj_stage(w_dram, name):
                    """projection + raw evict + sumsq; returns (qt tiles, ms tiles)"""
                    qt_cur = {}
                    ms_sb = {}
                    for pair, qh, pj in project(w_dram, name):
                        if qh == 0:
                            qt_cur[pair] = qtp.tile([128, T], F32, tag="qt",
                                                     name=f"qt{name}{pair}")
                            ms_sb[pair] = nsml.tile([2, T], F32, tag=f"ms{name}{pair}",
                                                    name=f"ms{name}{pair}")
                        qt_sb = qt_cur[pair]
                        sl = slice(qh * 1024, (qh + 1) * 1024)
                        nc.scalar.copy(qt_sb[:, sl], pj[:])
                        q2 = q2p.tile([128, 1024], F32R, tag="q2")
                        with nc.allow_low_precision(reason="fp32r rounding"):
                            nc.vector.tensor_mul(q2[:], qt_sb[:, sl], qt_sb[:, sl])
                        for qq in range(2):
                            ss = ps_sml.tile([2, 512], F32, tag="sml")
                            nc.tensor.matmul(ss[:], bd2[:], q2[:, qq * 512:(qq + 1) * 512],
                                             start=True, stop=True)
                            nc.vector.tensor_copy(
                                out=ms_sb[pair][:, qh * 1024 + qq * 512:
                                                qh * 1024 + (qq + 1) * 512],
                                in_=ss[:])
                    return qt_cur, ms_sb

                def norm_stage(qt_cur, ms_sb, wb, dest, name):
                    # rstd = (ms/64+eps)^-1/2 = exp(-0.5*ln(ms/64+eps))
                    rstds = {}
                    for pair in range(NPAIR):
                        nc.scalar.activation(ms_sb[pair][:], ms_sb[pair][:], Log,
                                             scale=1.0 / HD, bias=epsc[0:2, :])
                        rstd = nsml.tile([128, T], F32R, tag=f"rstd{pair}",
                                         name=f"rstd{name}{pair}")
                        nc.vector.memset(rstd[:, :].bitcast(F32), 0.0)
                        with nc.allow_low_precision(reason="fp32r rounding"):
                            nc.scalar.activation(rstd[0:2, :], ms_sb[pair][:], Exp,
                                                 scale=-0.5)
                        rstds[pair] = rstd
                    for pair in range(NPAIR):
                        for qh in range(QH):
                            sl = slice(qh * 1024, (qh + 1) * 1024)
                            rw = ps_big.tile([128, 1024], F32, tag="big")
                            for qq in range(2):
                                nc.tensor.matmul(
                                    rw[:, qq * 512:(qq + 1) * 512], wb[:],
                                    rstds[pair][:, qh * 1024 + qq * 512:
                                                qh * 1024 + (qq + 1) * 512],
                                    start=True, stop=True)
                            with nc.allow_low_precision(reason="fp32r rounding"):
                                for i in range(2):
                                    rows = slice(64 * i, 64 * i + 64)
                                    nc.vector.tensor_mul(
                                        dest[pair * 2 + i][rows, sl],
                                        qt_cur[pair][rows, sl], rw[rows, :])

                qt_q, ms_q = proj_stage(wq_s, "q")
                qt_k, ms_k = proj_stage(wk_s, "k")

                # ---- V: project to V^T then transpose into [V|1] tiles ----
                for pair, qh, pj in project(wv_s, "v"):
                    if qh == 0:
                        vt_sb = qtp.tile([128, T], HOT, tag="vt")
                        vt_cur = vt_sb
                    else:
                        vt_sb = vt_cur
                    nc.scalar.copy(vt_sb[:, qh * 1024:(qh + 1) * 1024], pj[:])
                    if qh == QH - 1:
                        for tt in range(TT):
                            if tt % 4 == 0:
                                vb_ps = ps_big.tile([128, 512], HOT, tag="big",
                                                    name="vb_ps")
                            j = tt % 4
                            nc.tensor.transpose(
                                vb_ps[:, j * 128:(j + 1) * 128],
                                vt_sb[:, tt * 128:(tt + 1) * 128], ident[:])
                            bl = vb_ps[:, j * 128:(j + 1) * 128]
                            with nc.allow_low_precision(reason="fp32r rounding"):
                                nc.vector.tensor_copy(
                                    out=vp[pair][:, tt, :, 0:64],
                                    in_=bl.rearrange("p (h d) -> p h d", h=2))

                norm_stage(qt_q, ms_q, wqb, qhat, "q")
                norm_stage(qt_k, ms_k, wkb, khat, "k")

            # scheduler fence: nothing from P2/P3 may be hoisted before P0/P1
            tc.no_sync_barrier()

            # =============== Phase 2+3: attention + output projection =============
            with ExitStack() as p23:
                ppool = p23.enter_context(tc.tile_pool(name="p", bufs=3))
                dntp = p23.enter_context(tc.tile_pool(name="dnt", bufs=2))
                dnp = p23.enter_context(tc.tile_pool(name="dn", bufs=1))
                ohpool = p23.enter_context(tc.tile_pool(name="ohp", bufs=1))
                outsbp = p23.enter_context(tc.tile_pool(name="outsb", bufs=2))
                wop = p23.enter_context(tc.tile_pool(name="wo", bufs=2))
                ps_sbig = p23.enter_context(
                    tc.tile_pool(name="pssbig", bufs=3, space="PSUM"))
                ps_o = p23.enter_context(
                    tc.tile_pool(name="pso", bufs=1, space="PSUM"))

                dn_all = dnp.tile([4, T], F32, tag="dn")
                ohp = [ohpool.tile([128, T], F32, tag=f"ohp{p}", name=f"ohp{p}")
                       for p in range(NPAIR)]
                ohr = [ohpool.tile([128, T], HOT, tag=f"ohr{p}", name=f"ohr{p}")
                       for p in range(NPAIR)]

                for h in range(HPC):
                    pair, i = h // 2, h % 2
                    Ks = khat[h]
                    Qs = qhat[h]
                    for qh in range(QH):
                        o_ps = ps_o.tile([128, 1024], F32, tag="o")
                        orows = slice(0, 65)
                        vcol = i
                        for kt in range(TT):
                            s_ps = ps_sbig.tile([128, 1024], F32, tag="sbig")
                            for qq in range(2):
                                nc.tensor.matmul(
                                    s_ps[:, qq * 512:(qq + 1) * 512],
                                    Ks[:, kt * 128:(kt + 1) * 128],
                                    Qs[:, qh * 1024 + qq * 512:qh * 1024 + (qq + 1) * 512],
                                    start=True, stop=True)
                            p_sb = ppool.tile([128, 1024], HOT, tag="p")
                            nc.scalar.activation(p_sb[:], s_ps[:], Exp, scale=0.125)
                            for qq in range(2):
                                nc.tensor.matmul(
                                    o_ps[orows, qq * 512:(qq + 1) * 512],
                                    vp[pair][:, kt, vcol, :],
                                    p_sb[:, qq * 512:(qq + 1) * 512],
                                    start=(kt == 0), stop=(kt == TT - 1))
                        # evict raw O^T rows + denominator row
                        sl = slice(qh * 1024, (qh + 1) * 1024)
                        dnt = dntp.tile([65, 1024], F32, tag="dnt")
                        nc.vector.tensor_copy(out=dnt[64:65, :], in_=o_ps[64:65, :])
                        nc.sync.dma_start(out=dn_all[h:h + 1, sl], in_=dnt[64:65, :])
                        if i == 0:
                            nc.vector.tensor_copy(out=ohp[pair][0:64, sl],
                                                  in_=o_ps[0:64, :])
                        else:
                            # cross-partition move: DVE to tmp rows 0:64, DMA remap
                            nc.vector.tensor_copy(out=dnt[0:64, :], in_=o_ps[0:64, :])
                            nc.sync.dma_start(out=ohp[pair][64:128, sl],
                                              in_=dnt[0:64, :])

                dnl = dnp.tile([4, T], F32, tag="dnl")
                nc.scalar.activation(dnl[:], dn_all[:], Log, scale=1.0)
                dnr = dnp.tile([128, T], F32R, tag="dnr")
                nc.vector.memset(dnr[:, :].bitcast(F32), 0.0)
                with nc.allow_low_precision(reason="fp32r rounding"):
                    nc.scalar.activation(dnr[0:4, :], dnl[:], Exp, scale=-1.0)
                for pair in range(NPAIR):
                    for qh in range(QH):
                        rb = ps_sbig.tile([128, 1024], F32, tag="sbig")
                        for qq in range(2):
                            nc.tensor.matmul(
                                rb[:, qq * 512:(qq + 1) * 512], sel[pair][:],
                                dnr[:, qh * 1024 + qq * 512:qh * 1024 + (qq + 1) * 512],
                                start=True, stop=True)
                        sl = slice(qh * 1024, (qh + 1) * 1024)
                        with nc.allow_low_precision(reason="fp32r rounding"):
                            nc.vector.tensor_mul(ohr[pair][:, sl], ohp[pair][:, sl], rb[:])

                # ---- output projection: Out^T = wo_s.T @ Ohat^T ----
                wo_sb = []
                for cp in range(NPAIR):
                    wt = wop.tile([128, D], HOT, tag="wo", name=f"wo{cp}")
                    nc.sync.dma_start(out=wt[:],
                                      in_=wo_s[cp * 128:(cp + 1) * 128, :].bitcast(HOT))
                    wo_sb.append(wt)
                for et in range(D // 128):
                    osb = outsbp.tile([128, T], F32, tag="outsb")
                    for tcn in range(T // 512):
                        ops = ps_sbig.tile([128, 512], F32, tag="sbig")
                        for cp in range(NPAIR):
                            nc.tensor.matmul(
                                ops[:], wo_sb[cp][:, et * 128:(et + 1) * 128],
                                ohr[cp][:, tcn * 512:(tcn + 1) * 512],
                                start=(cp == 0), stop=(cp == NPAIR - 1))
                        nc.vector.tensor_copy(out=osb[:, tcn * 512:(tcn + 1) * 512],
                                              in_=ops[:])
                    nc.sync.dma_start(out=outT[et * 128:(et + 1) * 128, :], in_=osb[:])

    nc.compile()
    return nc


def _get_compiled():
    global _COMPILED
    if _COMPILED is None:
        _COMPILED = _build()
    return _COMPILED


def _make_consts(q_norm_w, k_norm_w):
    ident = np.eye(128, dtype=np.float32)
    bd2 = np.zeros((128, 2), np.float32)
    bd2[0:64, 0] = 1.0
    bd2[64:128, 1] = 1.0
    wqb = np.zeros((128, 128), np.float32)
    wqb[0, 0:64] = q_norm_w
    wqb[1, 64:128] = q_norm_w
    wkb = np.zeros((128, 128), np.float32)
    wkb[0, 0:64] = k_norm_w
    wkb[1, 64:128] = k_norm_w
    sels = []
    for p in range(2):
        s = np.zeros((128, 128), np.float32)
        s[p * 2, 0:64] = 1.0
        s[p * 2 + 1, 64:128] = 1.0
        sels.append(s)
    onec = np.ones((128, 1), np.float32)
    return ident, bd2, wqb, wkb, sels, onec


def kernel(x, wq, wk, wv, wo, q_norm_w, k_norm_w):
    from concourse.bass_utils import run_bass_kernel_spmd

    global LAST_EXEC_NS
    if os.environ.get("BASS_TRACE"):
        _install_ntff_shim()

    x = np.asarray(x, dtype=np.float32)
    wq = np.asarray(wq, dtype=np.float32)
    wk = np.asarray(wk, dtype=np.float32)
    wv = np.asarray(wv, dtype=np.float32)
    wo = np.asarray(wo, dtype=np.float32)
    q_norm_w = np.asarray(q_norm_w, dtype=np.float32)
    k_norm_w = np.asarray(k_norm_w, dtype=np.float32)

    nc = _get_compiled()
    ident, bd2, wqb, wkb, sels, onec = _make_consts(q_norm_w, k_norm_w)
    if HOT_BF16:
        import ml_dtypes
        cast = lambda a: a.astype(ml_dtypes.bfloat16)
    else:
        cast = lambda a: a

    in_maps = []
    for c in range(N_CORES):
        b = c // 4
        hs = HPC * (c % 4)
        # head split in reference is strided: head h uses columns d*H + h
        perm = ((hs + np.arange(HPC))[:, None] + H * np.arange(HD)[None, :]).reshape(-1)
        in_maps.append({
            "xb": cast(np.ascontiguousarray(x[b])),
            "wq_s": cast(np.ascontiguousarray(wq[:, perm])),
            "wk_s": cast(np.ascontiguousarray(wk[:, perm])),
            "wv_s": cast(np.ascontiguousarray(wv[:, perm])),
            "wo_s": cast(np.ascontiguousarray(wo[hs * HD:(hs + HPC) * HD, :])),
            "ident": cast(ident), "bd2": bd2, "wqb": wqb, "wkb": wkb,
            "sel0": sels[0], "sel1": sels[1], "onec": cast(onec),
        })

    res = run_bass_kernel_spmd(nc, in_maps, core_ids=list(range(N_CORES)),
                               trace=bool(os.environ.get("BASS_TRACE")),
                               tmpdir=os.environ.get("BASS_TRACE_DIR"))
    LAST_EXEC_NS = res.exec_time_ns

    out = np.empty((B, T, D), dtype=np.float32)
    for b in range(B):
        acc = res.results[4 * b]["outT"].astype(np.float32).copy()
        for c in range(4 * b + 1, 4 * b + 4):
            acc += res.results[c]["outT"]
        out[b] = acc.T
    return out
